# revision 13
# baseline (speedup 1.0000x reference)
"""Trainium2 Bass kernel for nn_BiARMA (2-layer ARMA GNN, K=2 stacks, T=2).

Math: A = D^-1/2 C D^-1/2 (C = edge-count matrix, deg by dst).
norm[e] = dinv[src]*dinv[dst] factors, so every message-passing round is
a row-gather-accumulate of a pre-scaled node tensor; weights commute
with aggregation and run on the aggregated tensor.

Distribution: dst-nodes sharded over 8 cores.  Each core keeps a full
replica of the current pre-scaled node tensor in DRAM, split into two
halves by LOCAL node index so int16 gather indices reach every row.

Aggregation (the key difference vs the padded-reduce scheme): edges are
packed DENSELY per (dst block, half) into 128-row gather columns (only
~15% pad from rounding to 128, instead of ~77% from per-node max-width
padding).  The segmented sum is done on the PE: per column, a one-hot
"staircase" matrix St[e, d] = (dstlow[e] == d) is built on the DVE from
a precomputed per-edge dst table and an iota tile, and
psum[F, 128dst] += gv[128e, F]^T @ St accumulates the per-block
aggregate TRANSPOSED — which feeds the weight matmul as lhsT directly,
eliminating both the DVE segmented reduce and the PE transpose.

The per-round SWDGE descriptor count drops ~1.6x; descriptor generation
on the Pool engine is the dominant serial cost on HW.
"""

import os
import sys
from dataclasses import dataclass, field

import numpy as np

sys.path.insert(0, "/opt/trn_rl_repo")

P = 128


@dataclass
class Cfg:
    N: int = 50000
    E: int = 800000
    IN_C: int = 64
    HID_C: int = 64
    OUT_C: int = 32
    K: int = 2
    CORES: int = 8
    # gather-tile budget, BYTES per partition per wave-call
    group_budget_bytes: int = 32768
    # per-call column cap: cols*128 descriptors must stay well under the
    # 16384-descriptor SWDGE ring
    col_cap: int = int(os.environ.get("GNN_COL_CAP", "60"))

    @property
    def blocks(self):
        nb = (self.N // self.CORES + 2 + P - 1) // P
        return nb + (nb & 1)

    @property
    def NPC(self):
        return self.blocks * P

    @property
    def NBH(self):  # blocks in half A
        return self.blocks // 2

    @property
    def NHALF(self):  # local rows per half
        return self.NPC // 2

    @property
    def RH(self):  # rows per half replica (all cores' halves stacked)
        return self.CORES * self.NHALF

    @property
    def PAD_LOC(self):  # phantom row (value 0) inside each half replica
        return self.NHALF - 1


@dataclass
class Struct:
    ncA: list
    ncB: list
    a_cum: list
    b_cum: list
    TA: int
    TB: int
    idx16: np.ndarray = None
    stair: np.ndarray = None
    pid: np.ndarray = None
    dinv_pc: np.ndarray = None
    groups: dict = field(default_factory=dict)


def build_structure(edge_index: np.ndarray, cfg: Cfg) -> Struct:
    src = np.asarray(edge_index[0], dtype=np.int64)
    dst = np.asarray(edge_index[1], dtype=np.int64)
    N, CORES, NPC, NB = cfg.N, cfg.CORES, cfg.NPC, cfg.blocks
    NHALF = cfg.NHALF

    deg = np.bincount(dst, minlength=N)
    order = np.argsort(-deg, kind="stable")
    rank = np.arange(N)
    core_of = np.empty(N, np.int64)
    raw_local = np.empty(N, np.int64)
    core_of[order] = rank % CORES
    raw_local[order] = rank // CORES
    # skip local NHALF-1 so each half keeps a phantom (zero) pad row
    local_of = raw_local + (raw_local >= (NHALF - 1))
    assert local_of.max() < NPC - 1
    pid = core_of * NPC + local_of

    ecore = core_of[dst]
    dloc = local_of[dst]
    half = (local_of[src] >= NHALF).astype(np.int64)
    # row within the half replica [RH]
    sloc = core_of[src] * NHALF + (local_of[src] - half * NHALF)

    # per (half, core, block) edge counts -> shared per-block column counts
    cnt = np.zeros((2, CORES, NB), np.int64)
    blk_all = dloc // P
    for h in (0, 1):
        for c in range(CORES):
            m = (ecore == c) & (half == h)
            cnt[h, c] = np.bincount(blk_all[m], minlength=NB)
    ncA = np.maximum(1, -(-cnt[0].max(axis=0) // P)).tolist()
    ncB = np.maximum(1, -(-cnt[1].max(axis=0) // P)).tolist()
    a_cum = np.concatenate([[0], np.cumsum(ncA)]).astype(np.int64)
    b_cum = np.concatenate([[0], np.cumsum(ncB)]).astype(np.int64)
    TA, TB = int(a_cum[-1]), int(b_cum[-1])

    st = Struct(ncA=ncA, ncB=ncB, a_cum=a_cum.tolist(), b_cum=b_cum.tolist(),
                TA=TA, TB=TB, pid=pid)

    # host-side degree -> dinv per (core, partition, block); phantoms get 0
    dinv_n = np.where(deg > 0, deg.astype(np.float64) ** -0.5, 0.0)
    dinv_pc = np.zeros((CORES, P, NB), np.float32)
    dinv_pc[core_of, local_of % P, local_of // P] = dinv_n
    st.dinv_pc = dinv_pc

    # per-core edge placement into dense 128-row columns
    vidx = np.full((CORES, P, TA + TB), cfg.PAD_LOC, np.int16)
    vdst = np.zeros((CORES, P, TA + TB), np.float32)
    for c in range(CORES):
        for h, (cum, off) in ((0, (a_cum, 0)), (1, (b_cum, TA))):
            m = (ecore == c) & (half == h)
            dl, sl = dloc[m], sloc[m]
            o = np.argsort(dl, kind="stable")
            dl, sl = dl[o], sl[o]
            blk = dl // P
            cntb = np.bincount(blk, minlength=NB)
            starts = np.concatenate([[0], np.cumsum(cntb)])[:-1]
            pos = np.arange(dl.shape[0]) - starts[blk]
            col = off + cum[blk] + pos // P
            pp = pos % P
            vidx[c, pp, col] = sl
            vdst[c, pp, col] = (dl % P).astype(np.float32)

    # idx16 wrap for the dma_gather call layout:
    # flat i = col*128 + p -> (i%16, i//16), replicated per Q7 core group
    idx16 = np.empty((CORES, 16, (TA + TB) * 8), np.int16)
    for c in range(CORES):
        w = vidx[c].reshape(16, 8, TA + TB, order="F")
        idx16[c] = np.transpose(w, (0, 2, 1)).reshape(16, (TA + TB) * 8)
    st.idx16 = np.tile(idx16, (1, 8, 1))
    st.stair = vdst

    # gather groups (cap on per-call columns, both waves independently)
    G1 = cfg.K * cfg.HID_C
    G2 = cfg.K * cfg.OUT_C
    row_bytes = {1: cfg.IN_C * 4, 2: G1 * 2, 3: cfg.HID_C * 4, 4: G2 * 4}

    def make_groups(rb):
        cap = max(min(cfg.group_budget_bytes // rb, cfg.col_cap),
                  max(max(ncA), max(ncB)))
        groups = []
        for lo, hi in ((0, cfg.NBH), (cfg.NBH, NB)):
            b0 = lo
            while b0 < hi:
                b1 = b0
                ta = tb = 0
                while b1 < hi and (
                        (ta + ncA[b1] <= cap and tb + ncB[b1] <= cap)
                        or b1 == b0):
                    ta += ncA[b1]
                    tb += ncB[b1]
                    b1 += 1
                groups.append((b0, b1))
                b0 = b1
        return groups

    st.groups = {r: make_groups(row_bytes[r]) for r in (1, 2, 3, 4)}
    return st


def build_weight_inputs(inp: dict, cfg: Cfg) -> dict:
    K, IN_C, HID_C, OUT_C = cfg.K, cfg.IN_C, cfg.HID_C, cfg.OUT_C
    f4 = lambda a: np.ascontiguousarray(a, dtype=np.float32)

    rootw1 = np.transpose(inp["root_w1"][0], (1, 0, 2)).reshape(IN_C, K * HID_C)
    b1row = inp["b1"][0, :, 0, :].reshape(1, K * HID_C)
    initw1 = np.transpose(inp["init_w1"], (1, 0, 2)).reshape(IN_C, K * HID_C)
    w1bd = np.zeros((K * HID_C, K * HID_C), np.float32)
    for k in range(K):
        w1bd[k * HID_C:(k + 1) * HID_C, k * HID_C:(k + 1) * HID_C] = inp["w1"][0, k]

    # 0.5 absorbed: round-2 h-stage feeds the UNhalved stack sum into root2
    rootw2 = 0.5 * np.transpose(inp["root_w2"][0], (1, 0, 2)).reshape(HID_C, K * OUT_C)
    b2row = inp["b2"][0, :, 0, :].reshape(1, K * OUT_C)
    initw2 = np.transpose(inp["init_w2"], (1, 0, 2)).reshape(HID_C, K * OUT_C)
    w2bd = np.zeros((K * OUT_C, K * OUT_C), np.float32)
    for k in range(K):
        w2bd[k * OUT_C:(k + 1) * OUT_C, k * OUT_C:(k + 1) * OUT_C] = inp["w2"][0, k]

    return {
        "w_rootw1": f4(rootw1), "w_b1": f4(b1row), "w_initw1": f4(initw1),
        "w_w1bd": f4(w1bd), "w_rootw2": f4(rootw2), "w_b2": f4(b2row),
        "w_initw2": f4(initw2), "w_w2bd": f4(w2bd),
        "w_iota": np.arange(P, dtype=np.float32).reshape(1, P),
    }


def build_nc(cfg: Cfg, st: Struct):
    import concourse.bacc as bacc
    import concourse.mybir as mybir
    import concourse.tile as tile
    from concourse.masks import make_identity

    f32 = mybir.dt.float32
    bf16 = mybir.dt.bfloat16
    i16 = mybir.dt.int16
    Alu = mybir.AluOpType
    Act = mybir.ActivationFunctionType

    K, IN_C, HID_C, OUT_C = cfg.K, cfg.IN_C, cfg.HID_C, cfg.OUT_C
    G1 = K * HID_C   # 128
    G2 = K * OUT_C   # 64
    NB = cfg.blocks
    NBH = cfg.NBH
    NPC, NHALF, RH = cfg.NPC, cfg.NHALF, cfg.RH
    ncA, ncB = st.ncA, st.ncB
    TA, TB = st.TA, st.TB
    a_cum, b_cum = st.a_cum, st.b_cum
    WTOT = (TA + TB) * 8

    FW = {1: IN_C, 2: G1, 3: HID_C, 4: G2}   # gathered row width (elems)
    GW = {1: G1, 2: G1, 3: G2, 4: G2}        # matmul output width
    YDT = {1: f32, 2: bf16, 3: f32, 4: f32}  # payload dtype per round
    DSZ = {1: 4, 2: 2, 3: 4, 4: 4}

    nc = bacc.Bacc(
        "TRN2",
        target_bir_lowering=False,
        debug=False,
        num_devices=cfg.CORES,
        num_swdge_queues=int(os.environ.get("GNN_NQUEUES", "4")),
    )

    # ---- kernel I/O ----
    xs = nc.dram_tensor("xs", [NPC, IN_C], f32, kind="ExternalInput")
    idx16_d = nc.dram_tensor("idx16", [P, WTOT], i16, kind="ExternalInput")
    stair_d = nc.dram_tensor("stair", [P, TA + TB], f32, kind="ExternalInput")
    dinv_d = nc.dram_tensor("dinv", [P, NB], f32, kind="ExternalInput")
    w_rootw1 = nc.dram_tensor("w_rootw1", [IN_C, G1], f32, kind="ExternalInput")
    w_b1 = nc.dram_tensor("w_b1", [1, G1], f32, kind="ExternalInput")
    w_initw1 = nc.dram_tensor("w_initw1", [IN_C, G1], f32, kind="ExternalInput")
    w_w1bd = nc.dram_tensor("w_w1bd", [G1, G1], f32, kind="ExternalInput")
    w_rootw2 = nc.dram_tensor("w_rootw2", [HID_C, G2], f32, kind="ExternalInput")
    w_b2 = nc.dram_tensor("w_b2", [1, G2], f32, kind="ExternalInput")
    w_initw2 = nc.dram_tensor("w_initw2", [HID_C, G2], f32, kind="ExternalInput")
    w_w2bd = nc.dram_tensor("w_w2bd", [G2, G2], f32, kind="ExternalInput")
    w_iota = nc.dram_tensor("w_iota", [1, P], f32, kind="ExternalInput")
    out_d = nc.dram_tensor("out", [NPC, OUT_C], f32, kind="ExternalOutput")

    # ---- internal DRAM: per-half replicas + per-half AG inputs ----
    yA = {r: nc.dram_tensor(f"yA{r}", [RH, FW[r]], YDT[r], addr_space="Shared")
          for r in (1, 2, 3, 4)}
    yB = {r: nc.dram_tensor(f"yB{r}", [RH, FW[r]], YDT[r], addr_space="Shared")
          for r in (1, 2, 3, 4)}
    agA = {r: nc.dram_tensor(f"agA{r}", [NHALF, FW[r]], YDT[r])
           for r in (1, 2, 3, 4)}
    agB = {r: nc.dram_tensor(f"agB{r}", [NHALF, FW[r]], YDT[r])
           for r in (1, 2, 3, 4)}

    rg = [list(range(cfg.CORES))]

    # gather tile: sized in f32 elems, bitcast for bf16 rounds
    max_gt_f32 = 0
    for r in (1, 2, 3, 4):
        for (b0, b1) in st.groups[r]:
            nA = (a_cum[b1] - a_cum[b0]) * FW[r] * DSZ[r]
            nB = (b_cum[b1] - b_cum[b0]) * FW[r] * DSZ[r]
            max_gt_f32 = max(max_gt_f32, (max(nA, nB) + 3) // 4)

    NSEM = 8
    dsems = [nc.alloc_semaphore(f"gsem{i}") for i in range(NSEM)]
    sem_count = [0] * NSEM
    gidx = [0]

    max_round = int(os.environ.get("GNN_STAGE", "4"))
    skip_ag = bool(os.environ.get("GNN_SKIP_AG"))

    with tile.TileContext(nc) as tc:
        with (
            tc.tile_pool(name="const", bufs=1) as cpool,
            tc.tile_pool(name="work", bufs=3) as wpool,
            tc.tile_pool(name="stpool", bufs=2) as stpool,
            tc.tile_pool(name="ghpool", bufs=3) as ghpool,
            tc.tile_pool(name="psum", bufs=3, space="PSUM") as ppool,
            tc.tile_pool(name="spsum", bufs=2, space="PSUM") as sppool,
        ):
            # ---------- constants ----------
            ident = cpool.tile([P, P], f32)
            make_identity(nc, ident[:])

            def load_w(t, shape, tag):
                s = cpool.tile(list(shape), f32, tag=tag)
                nc.sync.dma_start(out=s[:], in_=t[:, :])
                return s

            rootw1_s = load_w(w_rootw1, (IN_C, G1), "w_rootw1")
            initw1_s = load_w(w_initw1, (IN_C, G1), "w_initw1")
            w1bd_s = load_w(w_w1bd, (G1, G1), "w_w1bd")
            rootw2_s = load_w(w_rootw2, (HID_C, G2), "w_rootw2")
            initw2_s = load_w(w_initw2, (HID_C, G2), "w_initw2")
            w2bd_s = load_w(w_w2bd, (G2, G2), "w_w2bd")
            b1_s = load_w(w_b1, (1, G1), "w_b1")
            b2_s = load_w(w_b2, (1, G2), "w_b2")
            iota_row = load_w(w_iota, (1, P), "w_iota")
            rhs_s = {1: initw1_s, 2: w1bd_s, 3: initw2_s, 4: w2bd_s}

            ones1 = cpool.tile([1, P], f32)
            nc.vector.memset(ones1[:], 1.0)
            b1rep = cpool.tile([P, G1], f32)
            b2rep = cpool.tile([P, G2], f32)
            bps = ppool.tile([P, G1], f32, tag="mmps")
            nc.tensor.matmul(bps[:], lhsT=ones1[:], rhs=b1_s[:], start=True, stop=True)
            nc.vector.tensor_copy(b1rep[:], bps[:])
            bps2 = ppool.tile([P, G2], f32, tag="mmps")
            nc.tensor.matmul(bps2[:], lhsT=ones1[:], rhs=b2_s[:], start=True, stop=True)
            nc.vector.tensor_copy(b2rep[:], bps2[:])
            # IOTA[p, j] = j  (outer product ones x iota_row)
            iota_sb = cpool.tile([P, P], f32)
            iops = ppool.tile([P, P], f32, tag="mmps")
            nc.tensor.matmul(iops[:], lhsT=ones1[:], rhs=iota_row[:], start=True, stop=True)
            nc.vector.tensor_copy(iota_sb[:], iops[:])
            iota_bf = cpool.tile([P, P], bf16)
            nc.vector.tensor_copy(iota_bf[:], iota_sb[:])

            # ---------- gather indices + staircase dst table ----------
            idx16_s = cpool.tile([P, WTOT], i16)
            nc.sync.dma_start(out=idx16_s[:], in_=idx16_d[:, :])
            stair_s = cpool.tile([P, TA + TB], f32)
            nc.sync.dma_start(out=stair_s[:], in_=stair_d[:, :])
            stair_bf = cpool.tile([P, TA + TB], bf16)
            nc.vector.tensor_copy(stair_bf[:], stair_s[:])

            # ---------- persistent per-round state ----------
            root1 = cpool.tile([P, NB, G1], f32)
            root2 = cpool.tile([P, NB, G2], f32)
            dinv = cpool.tile([P, NB], f32)
            dinvh = cpool.tile([P, NB], f32)

            # ---------- dinv (host) + roots + y1 (prolog) ----------
            with tc.tile_pool(name="prolog", bufs=1) as qpool:
                nc.sync.dma_start(out=dinv[:], in_=dinv_d[:, :])
                nc.vector.tensor_scalar_mul(dinvh[:], dinv[:], 0.5)

                x_s = qpool.tile([P, NB, IN_C], f32)
                for b in range(NB):
                    nc.sync.dma_start(
                        out=x_s[:, b, :], in_=xs[b * P:(b + 1) * P, :]
                    )
                for b in range(NB):
                    dcol = dinv[:, b:b + 1]
                    xT_ps = ppool.tile([IN_C, P], f32, tag="tps")
                    nc.tensor.transpose(xT_ps[:], x_s[:, b, :], ident[:])
                    xT = wpool.tile([IN_C, P], f32, tag="aggT")
                    nc.scalar.activation(xT[:], xT_ps[:], Act.Copy)
                    r1_ps = ppool.tile([P, G1], f32, tag="mmps")
                    nc.tensor.matmul(
                        r1_ps[:], lhsT=xT[:], rhs=rootw1_s[:], start=True, stop=True
                    )
                    nc.vector.tensor_add(root1[:, b, :], r1_ps[:], b1rep[:])
                    y1b = wpool.tile([P, IN_C], f32, tag="yout")
                    nc.scalar.activation(y1b[:], x_s[:, b, :], Act.Copy, scale=dcol)
                    if b < NBH:
                        nc.sync.dma_start(
                            out=agA[1][b * P:(b + 1) * P, :], in_=y1b[:]
                        )
                    else:
                        bb = b - NBH
                        nc.sync.dma_start(
                            out=agB[1][bb * P:(bb + 1) * P, :], in_=y1b[:]
                        )
                    if b == NBH - 1 and not skip_ag:
                        nc.gpsimd.collective_compute(
                            "AllGather", Alu.bypass, replica_groups=rg,
                            ins=[agA[1].ap().opt()], outs=[yA[1].ap().opt()],
                        )
            if not skip_ag:
                nc.gpsimd.collective_compute(
                    "AllGather", Alu.bypass, replica_groups=rg,
                    ins=[agB[1].ap().opt()], outs=[yB[1].ap().opt()],
                )

            # gather pool opened after the prolog pool closes
            with tc.tile_pool(name="gather", bufs=int(os.environ.get("GNN_GBUFS", "4"))) as gpool:
                def gather_wave(r, b0, b1, wave):
                    """prepare_only dma_gather of the A- or B-half columns of
                    blocks [b0,b1); returns (tile, (sem, value))."""
                    F = FW[r]
                    dt = YDT[r]
                    if wave == "A":
                        ncols = a_cum[b1] - a_cum[b0]
                        ix = idx16_s[:, a_cum[b0] * 8:a_cum[b1] * 8]
                        yv = yA[r]
                    else:
                        ncols = b_cum[b1] - b_cum[b0]
                        ix = idx16_s[:, (TA + b_cum[b0]) * 8:(TA + b_cum[b1]) * 8]
                        yv = yB[r]
                    n = ncols * P
                    gt = gpool.tile([P, max_gt_f32], f32, tag="gt")
                    gv = gt[:].bitcast(dt) if dt != f32 else gt[:]
                    out = gv[:, :ncols * F].rearrange("p (c f) -> p c f", f=F)
                    s = gidx[0] % NSEM
                    q = gidx[0] % nc.num_swdge_queues
                    gidx[0] += 1
                    sem_count[s] += 16
                    nc.gpsimd.dma_gather(
                        out_ap=out, in_ap=yv.ap(), idxs_ap=ix,
                        num_idxs=n, num_idxs_reg=n, elem_size=F,
                        single_packet=False, prepare_only=True, sem=dsems[s],
                        queue_num=q,
                    )
                    nc.gpsimd.trigger_dma(count=None, queue_num=q)
                    return gv, (dsems[s], sem_count[s])

                # ---------- 4 message-passing rounds ----------
                def build_st(b0, b1, wave):
                    """one-hot staircase tiles for all columns of the wave's
                    blocks [b0, b1): st[p, j, d] = (stair[p, col0+j] == d)"""
                    if wave == "A":
                        c0, c1 = a_cum[b0], a_cum[b1]
                        off = 0
                    else:
                        c0, c1 = b_cum[b0], b_cum[b1]
                        off = TA
                    ncols = c1 - c0
                    stm = stpool.tile([P, ncols * P], bf16, tag="stm")
                    stv = stm[:].rearrange("p (c d) -> p c d", d=P)
                    iob = iota_bf[:].rearrange("p (c d) -> p c d", c=1) \
                        .broadcast_to([P, ncols, P])
                    scl = stair_bf[:, off + c0:off + c1] \
                        .rearrange("p c -> p c ()").broadcast_to([P, ncols, P])
                    nc.vector.tensor_tensor(stv, iob, scl, Alu.is_equal)
                    return stv

                for r in (1, 2, 3, 4):
                    if r > max_round:
                        break
                    F = FW[r]
                    G = GW[r]
                    for (b0, b1) in st.groups[r]:
                        gvA, wA = gather_wave(r, b0, b1, "A")
                        gvB, wB = gather_wave(r, b0, b1, "B")
                        stA = build_st(b0, b1, "A")
                        stB = build_st(b0, b1, "B")
                        # bf16 views of the gathered payload: round 2 is
                        # already bf16; other rounds cast f32->bf16 on the
                        # Scalar engine, which also carries the DMA wait
                        if r == 2:
                            ghA, ghB = gvA, gvB
                            firstA, firstB = [True], [True]
                        else:
                            nca = (a_cum[b1] - a_cum[b0]) * F
                            ncb = (b_cum[b1] - b_cum[b0]) * F
                            ghA = ghpool.tile([P, nca], bf16, tag="gh")
                            cp = nc.scalar.activation(ghA[:], gvA[:, :nca], Act.Copy)
                            cp._wait_ge(*wA)
                            ghB = ghpool.tile([P, ncb], bf16, tag="gh")
                            cp = nc.scalar.activation(ghB[:], gvB[:, :ncb], Act.Copy)
                            cp._wait_ge(*wB)
                            ghA, ghB = ghA[:], ghB[:]
                            firstA, firstB = [False], [False]
                        for b in range(b0, b1):
                            dcol = dinv[:, b:b + 1]
                            ps = sppool.tile([F, P], f32, tag="stairps")
                            nmm = ncA[b] + ncB[b]
                            mi = 0
                            for wave, gh, stw, w, first, cum in (
                                ("A", ghA, stA, wA, firstA, a_cum),
                                ("B", ghB, stB, wB, firstB, b_cum),
                            ):
                                o0 = cum[b] - cum[b0]
                                ncols = (ncA if wave == "A" else ncB)[b]
                                for j in range(ncols):
                                    colg = o0 + j
                                    if first[0]:
                                        # PE-stream wait: matmul lowers to
                                        # LDWEIGHTS+MATMUL and the per-inst
                                        # wait does not guard the LDWEIGHTS
                                        # read of the gather tile
                                        nc.tensor.wait_ge(*w)
                                        first[0] = False
                                    nc.tensor.matmul(
                                        ps[:], lhsT=gh[:, colg * F:(colg + 1) * F],
                                        rhs=stw[:, colg, :],
                                        start=(mi == 0), stop=(mi == nmm - 1),
                                    )
                                    mi += 1
                            # ---- per-block postprocess (aggT = ps) ----
                            aggT = wpool.tile([F, P], f32, tag="aggT")
                            nc.scalar.activation(aggT[:], ps[:], Act.Copy)
                            mm_ps = ppool.tile([P, G], f32, tag="mmps")
                            nc.tensor.matmul(
                                mm_ps[:], lhsT=aggT[:], rhs=rhs_s[r][:],
                                start=True, stop=True,
                            )
                            root = root1 if r <= 2 else root2
                            t_sb = wpool.tile([P, G], f32, tag="tsb")
                            nc.vector.scalar_tensor_tensor(
                                t_sb[:], mm_ps[:], dcol, root[:, b, :],
                                op0=Alu.mult, op1=Alu.add,
                            )
                            if os.environ.get("GNN_DBG") == f"tsb{r}":
                                nc.sync.dma_start(
                                    out=out_d[b * P:(b + 1) * P, :],
                                    in_=t_sb[:, :OUT_C],
                                )

                            def store_y(yo, rr):
                                if b < NBH:
                                    nc.sync.dma_start(
                                        out=agA[rr][b * P:(b + 1) * P, :], in_=yo[:]
                                    )
                                else:
                                    bb = b - NBH
                                    nc.sync.dma_start(
                                        out=agB[rr][bb * P:(bb + 1) * P, :], in_=yo[:]
                                    )

                            if r == 1:
                                yo = wpool.tile([P, G1], YDT[2], tag="yout")
                                nc.scalar.activation(yo[:], t_sb[:], Act.Relu, scale=dcol)
                                store_y(yo, 2)
                            elif r == 2:
                                out1 = wpool.tile([P, G1], f32, tag="out1")
                                nc.scalar.activation(out1[:], t_sb[:], Act.Relu)
                                hsum = wpool.tile([P, HID_C], f32, tag="hsum")
                                nc.vector.tensor_add(
                                    hsum[:], out1[:, :HID_C], out1[:, HID_C:]
                                )
                                yo = wpool.tile([P, HID_C], f32, tag="yout2")
                                nc.scalar.activation(
                                    yo[:], hsum[:], Act.Copy, scale=dinvh[:, b:b + 1]
                                )
                                store_y(yo, 3)
                                hT_ps = ppool.tile([HID_C, P], f32, tag="tps")
                                nc.tensor.transpose(hT_ps[:], hsum[:], ident[:])
                                hT = wpool.tile([HID_C, P], f32, tag="aggT2")
                                nc.scalar.activation(hT[:], hT_ps[:], Act.Copy)
                                r2_ps = ppool.tile([P, G2], f32, tag="mmps")
                                nc.tensor.matmul(
                                    r2_ps[:], lhsT=hT[:], rhs=rootw2_s[:],
                                    start=True, stop=True,
                                )
                                nc.vector.tensor_add(root2[:, b, :], r2_ps[:], b2rep[:])
                            elif r == 3:
                                yo = wpool.tile([P, G2], f32, tag="yout")
                                nc.scalar.activation(yo[:], t_sb[:], Act.Relu, scale=dcol)
                                store_y(yo, 4)
                            else:
                                ofin = wpool.tile([P, G2], f32, tag="out1")
                                nc.scalar.activation(ofin[:], t_sb[:], Act.Relu)
                                msum = wpool.tile([P, OUT_C], f32, tag="hsum")
                                nc.vector.tensor_add(
                                    msum[:], ofin[:, :OUT_C], ofin[:, OUT_C:]
                                )
                                yo = wpool.tile([P, OUT_C], f32, tag="yout")
                                nc.scalar.activation(yo[:], msum[:], Act.Copy, scale=0.5)
                                nc.sync.dma_start(
                                    out=out_d[b * P:(b + 1) * P, :], in_=yo[:]
                                )
                        # fire AG_A as soon as the half-A output shard exists
                        if b1 == NBH and r < 4 and r < max_round and not skip_ag:
                            nc.gpsimd.collective_compute(
                                "AllGather", Alu.bypass, replica_groups=rg,
                                ins=[agA[r + 1].ap().opt()],
                                outs=[yA[r + 1].ap().opt()],
                            )
                    if r < 4 and r < max_round and not skip_ag:
                        nc.gpsimd.collective_compute(
                            "AllGather", Alu.bypass, replica_groups=rg,
                            ins=[agB[r + 1].ap().opt()], outs=[yB[r + 1].ap().opt()],
                        )

    nc.compile()
    return nc


def build_in_maps(inputs: dict, cfg: Cfg, st: Struct) -> list:
    x = np.asarray(inputs["x"], dtype=np.float32)
    wmap = build_weight_inputs(inputs, cfg)
    in_maps = []
    for c in range(cfg.CORES):
        xs = np.zeros((cfg.NPC, cfg.IN_C), np.float32)
        mine = np.nonzero(st.pid // cfg.NPC == c)[0]
        loc = st.pid[mine] % cfg.NPC
        xs[loc] = x[mine]
        m = {
            "xs": xs,
            "idx16": np.ascontiguousarray(st.idx16[c]),
            "stair": np.ascontiguousarray(st.stair[c]),
            "dinv": np.ascontiguousarray(st.dinv_pc[c]),
        }
        m.update(wmap)
        in_maps.append(m)
    return in_maps


def assemble_output(results: list, cfg: Cfg, st: Struct) -> np.ndarray:
    full = np.concatenate(
        [np.asarray(results[c]["out"]) for c in range(cfg.CORES)], axis=0
    )
    return np.ascontiguousarray(full[st.pid]).astype(np.float32)


def kernel(**inputs) -> np.ndarray:
    from concourse.bass_utils import run_bass_kernel_spmd

    cfg = Cfg()
    st = build_structure(np.asarray(inputs["edge_index"]), cfg)
    nc = build_nc(cfg, st)
    in_maps = build_in_maps(inputs, cfg, st)
    res = run_bass_kernel_spmd(nc, in_maps, core_ids=list(range(cfg.CORES)))
    return assemble_output(res.results, cfg, st)


if __name__ == "__main__":
    pass


# revision 19
# speedup vs baseline: 1.5885x; 1.5885x over previous
"""Trainium2 Bass kernel for nn_BiARMA (2-layer ARMA GNN, K=2 stacks, T=2).

Math: A = D^-1/2 C D^-1/2 (C = edge-count matrix, deg by dst).
norm[e] = dinv[src]*dinv[dst] factors, so every message-passing round is
a row-gather-accumulate of a pre-scaled node tensor; weights commute
with aggregation and run on the aggregated tensor.

Distribution: dst-nodes sharded over 8 cores.  Each core keeps a full
replica of the current pre-scaled node tensor in DRAM, split into two
halves by LOCAL node index so int16 gather indices reach every row.

Aggregation (the key difference vs the padded-reduce scheme): edges are
packed DENSELY per (dst block, half) into 128-row gather columns (only
~15% pad from rounding to 128, instead of ~77% from per-node max-width
padding).  The segmented sum is done on the PE: per column, a one-hot
"staircase" matrix St[e, d] = (dstlow[e] == d) is built on the DVE from
a precomputed per-edge dst table and an iota tile, and
psum[F, 128dst] += gv[128e, F]^T @ St accumulates the per-block
aggregate TRANSPOSED — which feeds the weight matmul as lhsT directly,
eliminating both the DVE segmented reduce and the PE transpose.

The per-round SWDGE descriptor count drops ~1.6x; descriptor generation
on the Pool engine is the dominant serial cost on HW.
"""

import os
import sys
from dataclasses import dataclass, field

import numpy as np

sys.path.insert(0, "/opt/trn_rl_repo")

P = 128


@dataclass
class Cfg:
    N: int = 50000
    E: int = 800000
    IN_C: int = 64
    HID_C: int = 64
    OUT_C: int = 32
    K: int = 2
    CORES: int = 8
    # gather-tile budget, BYTES per partition per wave-call
    group_budget_bytes: int = 32768
    # per-call column cap: cols*128 descriptors must stay well under the
    # 16384-descriptor SWDGE ring
    col_cap: int = int(os.environ.get("GNN_COL_CAP", "60"))

    @property
    def blocks(self):
        nb = (self.N // self.CORES + 2 + P - 1) // P
        return nb + (nb & 1)

    @property
    def NPC(self):
        return self.blocks * P

    @property
    def NBH(self):  # blocks in half A
        return self.blocks // 2

    @property
    def NHALF(self):  # local rows per half
        return self.NPC // 2

    @property
    def RH(self):  # rows per half replica (all cores' halves stacked)
        return self.CORES * self.NHALF

    @property
    def PAD_LOC(self):  # phantom row (value 0) inside each half replica
        return self.NHALF - 1


@dataclass
class Struct:
    ncA: list
    ncB: list
    a_cum: list
    b_cum: list
    TA: int
    TB: int
    idx16: np.ndarray = None
    stair: np.ndarray = None
    pid: np.ndarray = None
    dinv_pc: np.ndarray = None
    groups: dict = field(default_factory=dict)


def build_structure(edge_index: np.ndarray, cfg: Cfg) -> Struct:
    src = np.asarray(edge_index[0], dtype=np.int64)
    dst = np.asarray(edge_index[1], dtype=np.int64)
    N, CORES, NPC, NB = cfg.N, cfg.CORES, cfg.NPC, cfg.blocks
    NHALF = cfg.NHALF

    deg = np.bincount(dst, minlength=N)
    order = np.argsort(-deg, kind="stable")
    rank = np.arange(N)
    core_of = np.empty(N, np.int64)
    raw_local = np.empty(N, np.int64)
    core_of[order] = rank % CORES
    raw_local[order] = rank // CORES
    # skip local NHALF-1 so each half keeps a phantom (zero) pad row
    local_of = raw_local + (raw_local >= (NHALF - 1))
    assert local_of.max() < NPC - 1
    pid = core_of * NPC + local_of

    ecore = core_of[dst]
    dloc = local_of[dst]
    half = (local_of[src] >= NHALF).astype(np.int64)
    # row within the half replica [RH]
    sloc = core_of[src] * NHALF + (local_of[src] - half * NHALF)

    # per (half, core, block) edge counts -> shared per-block column counts
    cnt = np.zeros((2, CORES, NB), np.int64)
    blk_all = dloc // P
    for h in (0, 1):
        for c in range(CORES):
            m = (ecore == c) & (half == h)
            cnt[h, c] = np.bincount(blk_all[m], minlength=NB)
    ncA = np.maximum(1, -(-cnt[0].max(axis=0) // P)).tolist()
    ncB = np.maximum(1, -(-cnt[1].max(axis=0) // P)).tolist()
    a_cum = np.concatenate([[0], np.cumsum(ncA)]).astype(np.int64)
    b_cum = np.concatenate([[0], np.cumsum(ncB)]).astype(np.int64)
    TA, TB = int(a_cum[-1]), int(b_cum[-1])

    st = Struct(ncA=ncA, ncB=ncB, a_cum=a_cum.tolist(), b_cum=b_cum.tolist(),
                TA=TA, TB=TB, pid=pid)

    # host-side degree -> dinv per (core, partition, block); phantoms get 0
    dinv_n = np.where(deg > 0, deg.astype(np.float64) ** -0.5, 0.0)
    dinv_pc = np.zeros((CORES, P, NB), np.float32)
    dinv_pc[core_of, local_of % P, local_of // P] = dinv_n
    st.dinv_pc = dinv_pc

    # per-core edge placement into dense 128-row columns
    vidx = np.full((CORES, P, TA + TB), cfg.PAD_LOC, np.int16)
    vdst = np.zeros((CORES, P, TA + TB), np.float32)
    for c in range(CORES):
        for h, (cum, off) in ((0, (a_cum, 0)), (1, (b_cum, TA))):
            m = (ecore == c) & (half == h)
            dl, sl = dloc[m], sloc[m]
            o = np.argsort(dl, kind="stable")
            dl, sl = dl[o], sl[o]
            blk = dl // P
            cntb = np.bincount(blk, minlength=NB)
            starts = np.concatenate([[0], np.cumsum(cntb)])[:-1]
            pos = np.arange(dl.shape[0]) - starts[blk]
            col = off + cum[blk] + pos // P
            pp = pos % P
            vidx[c, pp, col] = sl
            vdst[c, pp, col] = (dl % P).astype(np.float32)

    # idx16 wrap for the dma_gather call layout:
    # flat i = col*128 + p -> (i%16, i//16), replicated per Q7 core group
    idx16 = np.empty((CORES, 16, (TA + TB) * 8), np.int16)
    for c in range(CORES):
        w = vidx[c].reshape(16, 8, TA + TB, order="F")
        idx16[c] = np.transpose(w, (0, 2, 1)).reshape(16, (TA + TB) * 8)
    st.idx16 = np.tile(idx16, (1, 8, 1))
    st.stair = vdst

    # gather groups (cap on per-call columns, both waves independently)
    G1 = cfg.K * cfg.HID_C
    G2 = cfg.K * cfg.OUT_C
    row_bytes = {1: cfg.IN_C * 4, 2: G1 * 2, 3: cfg.HID_C * 4, 4: G2 * 4}

    def make_groups(rb):
        cap = max(min(cfg.group_budget_bytes // rb, cfg.col_cap),
                  max(max(ncA), max(ncB)))
        groups = []
        for lo, hi in ((0, cfg.NBH), (cfg.NBH, NB)):
            b0 = lo
            while b0 < hi:
                b1 = b0
                ta = tb = 0
                while b1 < hi and (
                        (ta + ncA[b1] <= cap and tb + ncB[b1] <= cap)
                        or b1 == b0):
                    ta += ncA[b1]
                    tb += ncB[b1]
                    b1 += 1
                groups.append((b0, b1))
                b0 = b1
        return groups

    st.groups = {r: make_groups(row_bytes[r]) for r in (1, 2, 3, 4)}
    return st


def build_weight_inputs(inp: dict, cfg: Cfg) -> dict:
    K, IN_C, HID_C, OUT_C = cfg.K, cfg.IN_C, cfg.HID_C, cfg.OUT_C
    f4 = lambda a: np.ascontiguousarray(a, dtype=np.float32)

    rootw1 = np.transpose(inp["root_w1"][0], (1, 0, 2)).reshape(IN_C, K * HID_C)
    b1row = inp["b1"][0, :, 0, :].reshape(1, K * HID_C)
    initw1 = np.transpose(inp["init_w1"], (1, 0, 2)).reshape(IN_C, K * HID_C)
    w1bd = np.zeros((K * HID_C, K * HID_C), np.float32)
    for k in range(K):
        w1bd[k * HID_C:(k + 1) * HID_C, k * HID_C:(k + 1) * HID_C] = inp["w1"][0, k]

    # 0.5 absorbed: round-2 h-stage feeds the UNhalved stack sum into root2
    rootw2 = 0.5 * np.transpose(inp["root_w2"][0], (1, 0, 2)).reshape(HID_C, K * OUT_C)
    b2row = inp["b2"][0, :, 0, :].reshape(1, K * OUT_C)
    initw2 = np.transpose(inp["init_w2"], (1, 0, 2)).reshape(HID_C, K * OUT_C)
    w2bd = np.zeros((K * OUT_C, K * OUT_C), np.float32)
    for k in range(K):
        w2bd[k * OUT_C:(k + 1) * OUT_C, k * OUT_C:(k + 1) * OUT_C] = inp["w2"][0, k]

    return {
        "w_rootw1": f4(rootw1), "w_b1": f4(b1row), "w_initw1": f4(initw1),
        "w_w1bd": f4(w1bd), "w_rootw2": f4(rootw2), "w_b2": f4(b2row),
        "w_initw2": f4(initw2), "w_w2bd": f4(w2bd),
        "w_iota": np.arange(P, dtype=np.float32).reshape(1, P),
    }


def build_nc(cfg: Cfg, st: Struct):
    import concourse.bacc as bacc
    import concourse.mybir as mybir
    import concourse.tile as tile
    from concourse.masks import make_identity

    f32 = mybir.dt.float32
    bf16 = mybir.dt.bfloat16
    i16 = mybir.dt.int16
    Alu = mybir.AluOpType
    Act = mybir.ActivationFunctionType

    K, IN_C, HID_C, OUT_C = cfg.K, cfg.IN_C, cfg.HID_C, cfg.OUT_C
    G1 = K * HID_C   # 128
    G2 = K * OUT_C   # 64
    NB = cfg.blocks
    NBH = cfg.NBH
    NPC, NHALF, RH = cfg.NPC, cfg.NHALF, cfg.RH
    ncA, ncB = st.ncA, st.ncB
    TA, TB = st.TA, st.TB
    a_cum, b_cum = st.a_cum, st.b_cum
    WTOT = (TA + TB) * 8

    FW = {1: IN_C, 2: G1, 3: HID_C, 4: G2}   # gathered row width (elems)
    GW = {1: G1, 2: G1, 3: G2, 4: G2}        # matmul output width
    YDT = {1: f32, 2: bf16, 3: f32, 4: f32}  # payload dtype per round
    DSZ = {1: 4, 2: 2, 3: 4, 4: 4}

    nc = bacc.Bacc(
        "TRN2",
        target_bir_lowering=False,
        debug=False,
        num_devices=cfg.CORES,
        num_swdge_queues=int(os.environ.get("GNN_NQUEUES", "4")),
    )

    # ---- kernel I/O ----
    xs = nc.dram_tensor("xs", [NPC, IN_C], f32, kind="ExternalInput")
    idx16_d = nc.dram_tensor("idx16", [P, WTOT], i16, kind="ExternalInput")
    stair_d = nc.dram_tensor("stair", [P, TA + TB], f32, kind="ExternalInput")
    dinv_d = nc.dram_tensor("dinv", [P, NB], f32, kind="ExternalInput")
    w_rootw1 = nc.dram_tensor("w_rootw1", [IN_C, G1], f32, kind="ExternalInput")
    w_b1 = nc.dram_tensor("w_b1", [1, G1], f32, kind="ExternalInput")
    w_initw1 = nc.dram_tensor("w_initw1", [IN_C, G1], f32, kind="ExternalInput")
    w_w1bd = nc.dram_tensor("w_w1bd", [G1, G1], f32, kind="ExternalInput")
    w_rootw2 = nc.dram_tensor("w_rootw2", [HID_C, G2], f32, kind="ExternalInput")
    w_b2 = nc.dram_tensor("w_b2", [1, G2], f32, kind="ExternalInput")
    w_initw2 = nc.dram_tensor("w_initw2", [HID_C, G2], f32, kind="ExternalInput")
    w_w2bd = nc.dram_tensor("w_w2bd", [G2, G2], f32, kind="ExternalInput")
    w_iota = nc.dram_tensor("w_iota", [1, P], f32, kind="ExternalInput")
    out_d = nc.dram_tensor("out", [NPC, OUT_C], f32, kind="ExternalOutput")

    # ---- internal DRAM: per-half replicas + per-half AG inputs ----
    yA = {r: nc.dram_tensor(f"yA{r}", [RH, FW[r]], YDT[r], addr_space="Shared")
          for r in (1, 2, 3, 4)}
    yB = {r: nc.dram_tensor(f"yB{r}", [RH, FW[r]], YDT[r], addr_space="Shared")
          for r in (1, 2, 3, 4)}
    agA = {r: nc.dram_tensor(f"agA{r}", [NHALF, FW[r]], YDT[r])
           for r in (1, 2, 3, 4)}
    agB = {r: nc.dram_tensor(f"agB{r}", [NHALF, FW[r]], YDT[r])
           for r in (1, 2, 3, 4)}

    rg = [list(range(cfg.CORES))]

    # gather tile: sized in f32 elems, bitcast for bf16 rounds
    max_gt_f32 = 0
    for r in (1, 2, 3, 4):
        for (b0, b1) in st.groups[r]:
            nA = (a_cum[b1] - a_cum[b0]) * FW[r] * DSZ[r]
            nB = (b_cum[b1] - b_cum[b0]) * FW[r] * DSZ[r]
            max_gt_f32 = max(max_gt_f32, (max(nA, nB) + 3) // 4)

    NSEM = 8
    dsems = [nc.alloc_semaphore(f"gsem{i}") for i in range(NSEM)]
    sem_count = [0] * NSEM
    gidx = [0]

    max_round = int(os.environ.get("GNN_STAGE", "4"))
    skip_ag = bool(os.environ.get("GNN_SKIP_AG"))

    with tile.TileContext(nc) as tc:
        with (
            tc.tile_pool(name="const", bufs=1) as cpool,
            tc.tile_pool(name="work", bufs=3) as wpool,
            tc.tile_pool(name="stpool", bufs=2) as stpool,
            tc.tile_pool(name="ghpool", bufs=3) as ghpool,
            tc.tile_pool(name="psum", bufs=3, space="PSUM") as ppool,
            tc.tile_pool(name="spsum", bufs=2, space="PSUM") as sppool,
        ):
            # ---------- constants ----------
            ident = cpool.tile([P, P], f32)
            make_identity(nc, ident[:])

            def load_w(t, shape, tag):
                s = cpool.tile(list(shape), f32, tag=tag)
                nc.sync.dma_start(out=s[:], in_=t[:, :])
                return s

            rootw1_s = load_w(w_rootw1, (IN_C, G1), "w_rootw1")
            initw1_s = load_w(w_initw1, (IN_C, G1), "w_initw1")
            w1bd_s = load_w(w_w1bd, (G1, G1), "w_w1bd")
            rootw2_s = load_w(w_rootw2, (HID_C, G2), "w_rootw2")
            initw2_s = load_w(w_initw2, (HID_C, G2), "w_initw2")
            w2bd_s = load_w(w_w2bd, (G2, G2), "w_w2bd")
            b1_s = load_w(w_b1, (1, G1), "w_b1")
            b2_s = load_w(w_b2, (1, G2), "w_b2")
            iota_row = load_w(w_iota, (1, P), "w_iota")
            rhs_s = {1: initw1_s, 2: w1bd_s, 3: initw2_s, 4: w2bd_s}

            ones1 = cpool.tile([1, P], f32)
            nc.vector.memset(ones1[:], 1.0)
            b1rep = cpool.tile([P, G1], f32)
            b2rep = cpool.tile([P, G2], f32)
            bps = ppool.tile([P, G1], f32, tag="mmps")
            nc.tensor.matmul(bps[:], lhsT=ones1[:], rhs=b1_s[:], start=True, stop=True)
            nc.vector.tensor_copy(b1rep[:], bps[:])
            bps2 = ppool.tile([P, G2], f32, tag="mmps")
            nc.tensor.matmul(bps2[:], lhsT=ones1[:], rhs=b2_s[:], start=True, stop=True)
            nc.vector.tensor_copy(b2rep[:], bps2[:])
            # IOTA[p, j] = j  (outer product ones x iota_row)
            iota_sb = cpool.tile([P, P], f32)
            iops = ppool.tile([P, P], f32, tag="mmps")
            nc.tensor.matmul(iops[:], lhsT=ones1[:], rhs=iota_row[:], start=True, stop=True)
            nc.vector.tensor_copy(iota_sb[:], iops[:])
            iota_bf = cpool.tile([P, P], bf16)
            nc.vector.tensor_copy(iota_bf[:], iota_sb[:])

            # ---------- gather indices + staircase dst table ----------
            idx16_s = cpool.tile([P, WTOT], i16)
            nc.sync.dma_start(out=idx16_s[:], in_=idx16_d[:, :])
            stair_s = cpool.tile([P, TA + TB], f32)
            nc.sync.dma_start(out=stair_s[:], in_=stair_d[:, :])
            stair_bf = cpool.tile([P, TA + TB], bf16)
            nc.vector.tensor_copy(stair_bf[:], stair_s[:])

            # ---------- persistent per-round state ----------
            root1 = cpool.tile([P, NB, G1], f32)
            root2 = cpool.tile([P, NB, G2], f32)
            dinv = cpool.tile([P, NB], f32)
            dinvh = cpool.tile([P, NB], f32)

            # ---------- dinv (host) + roots + y1 (prolog) ----------
            with tc.tile_pool(name="prolog", bufs=4) as qpool:
                nc.sync.dma_start(out=dinv[:], in_=dinv_d[:, :])
                nc.vector.tensor_scalar_mul(dinvh[:], dinv[:], 0.5)

                for b in range(NB):
                    x_b = qpool.tile([P, IN_C], f32, tag="xb")
                    nc.sync.dma_start(
                        out=x_b[:], in_=xs[b * P:(b + 1) * P, :]
                    )
                    dcol = dinv[:, b:b + 1]
                    xT_ps = ppool.tile([IN_C, P], f32, tag="tps")
                    nc.tensor.transpose(xT_ps[:], x_b[:], ident[:])
                    xT = wpool.tile([IN_C, P], f32, tag="aggT")
                    nc.scalar.activation(xT[:], xT_ps[:], Act.Copy)
                    r1_ps = ppool.tile([P, G1], f32, tag="mmps")
                    nc.tensor.matmul(
                        r1_ps[:], lhsT=xT[:], rhs=rootw1_s[:], start=True, stop=True
                    )
                    nc.vector.tensor_add(root1[:, b, :], r1_ps[:], b1rep[:])
                    y1b = wpool.tile([P, IN_C], f32, tag="yout")
                    nc.scalar.activation(y1b[:], x_b[:], Act.Copy, scale=dcol)
                    if b < NBH:
                        nc.sync.dma_start(
                            out=agA[1][b * P:(b + 1) * P, :], in_=y1b[:]
                        )
                    else:
                        bb = b - NBH
                        nc.sync.dma_start(
                            out=agB[1][bb * P:(bb + 1) * P, :], in_=y1b[:]
                        )
                    if b == NBH - 1 and not skip_ag:
                        nc.gpsimd.collective_compute(
                            "AllGather", Alu.bypass, replica_groups=rg,
                            ins=[agA[1].ap().opt()], outs=[yA[1].ap().opt()],
                        )
            if not skip_ag:
                nc.gpsimd.collective_compute(
                    "AllGather", Alu.bypass, replica_groups=rg,
                    ins=[agB[1].ap().opt()], outs=[yB[1].ap().opt()],
                )

            # gather pool opened after the prolog pool closes
            with tc.tile_pool(name="gather", bufs=int(os.environ.get("GNN_GBUFS", "5"))) as gpool:
                def gather_wave(r, b0, b1, wave, q):
                    """prepare_only dma_gather of the A- or B-half columns of
                    blocks [b0,b1); returns (tile, (sem, value))."""
                    F = FW[r]
                    dt = YDT[r]
                    if wave == "A":
                        ncols = a_cum[b1] - a_cum[b0]
                        ix = idx16_s[:, a_cum[b0] * 8:a_cum[b1] * 8]
                        yv = yA[r]
                    else:
                        ncols = b_cum[b1] - b_cum[b0]
                        ix = idx16_s[:, (TA + b_cum[b0]) * 8:(TA + b_cum[b1]) * 8]
                        yv = yB[r]
                    n = ncols * P
                    gt = gpool.tile([P, max_gt_f32], f32, tag="gt")
                    gv = gt[:].bitcast(dt) if dt != f32 else gt[:]
                    out = gv[:, :ncols * F].rearrange("p (c f) -> p c f", f=F)
                    s = gidx[0] % NSEM
                    gidx[0] += 1
                    sem_count[s] += 16
                    nc.gpsimd.dma_gather(
                        out_ap=out, in_ap=yv.ap(), idxs_ap=ix,
                        num_idxs=n, num_idxs_reg=n, elem_size=F,
                        single_packet=False, prepare_only=True, sem=dsems[s],
                        queue_num=q,
                    )
                    nc.gpsimd.trigger_dma(count=None, queue_num=q)
                    return gv, (dsems[s], sem_count[s])

                # ---------- 4 message-passing rounds ----------
                def build_st(b0, b1, wave):
                    """one-hot staircase tiles for all columns of the wave's
                    blocks [b0, b1): st[p, j, d] = (stair[p, col0+j] == d)"""
                    if wave == "A":
                        c0, c1 = a_cum[b0], a_cum[b1]
                        off = 0
                    else:
                        c0, c1 = b_cum[b0], b_cum[b1]
                        off = TA
                    ncols = c1 - c0
                    stm = stpool.tile([P, ncols * P], bf16, tag="stm")
                    stv = stm[:].rearrange("p (c d) -> p c d", d=P)
                    iob = iota_bf[:].rearrange("p (c d) -> p c d", c=1) \
                        .broadcast_to([P, ncols, P])
                    scl = stair_bf[:, off + c0:off + c1] \
                        .rearrange("p c -> p c ()").broadcast_to([P, ncols, P])
                    nc.vector.tensor_tensor(stv, iob, scl, Alu.is_equal)
                    return stv

                for r in (1, 2, 3, 4):
                    if r > max_round:
                        break
                    F = FW[r]
                    G = GW[r]
                    for gi, (b0, b1) in enumerate(st.groups[r]):
                        gvA, wA = gather_wave(r, b0, b1, "A", gidx[0] % 4)
                        gvB, wB = gather_wave(r, b0, b1, "B", gidx[0] % 4)
                        stA = build_st(b0, b1, "A")
                        stB = build_st(b0, b1, "B")
                        # bf16 views of the gathered payload: round 2 is
                        # already bf16; other rounds cast f32->bf16 on the
                        # Scalar engine, which also carries the DMA wait
                        if r == 2:
                            ghA, ghB = gvA, gvB
                            firstA, firstB = [True], [True]
                        else:
                            nca = (a_cum[b1] - a_cum[b0]) * F
                            ncb = (b_cum[b1] - b_cum[b0]) * F
                            ghA = ghpool.tile([P, nca], bf16, tag="gh")
                            cp = nc.scalar.activation(ghA[:], gvA[:, :nca], Act.Copy)
                            cp._wait_ge(*wA)
                            ghB = ghpool.tile([P, ncb], bf16, tag="gh")
                            cp = nc.scalar.activation(ghB[:], gvB[:, :ncb], Act.Copy)
                            cp._wait_ge(*wB)
                            ghA, ghB = ghA[:], ghB[:]
                            firstA, firstB = [False], [False]
                        for b in range(b0, b1):
                            dcol = dinv[:, b:b + 1]
                            ps = sppool.tile([F, P], f32, tag="stairps")
                            nmm = ncA[b] + ncB[b]
                            mi = 0
                            for wave, gh, stw, w, first, cum in (
                                ("A", ghA, stA, wA, firstA, a_cum),
                                ("B", ghB, stB, wB, firstB, b_cum),
                            ):
                                o0 = cum[b] - cum[b0]
                                ncols = (ncA if wave == "A" else ncB)[b]
                                for j in range(ncols):
                                    colg = o0 + j
                                    if first[0]:
                                        # PE-stream wait: matmul lowers to
                                        # LDWEIGHTS+MATMUL and the per-inst
                                        # wait does not guard the LDWEIGHTS
                                        # read of the gather tile
                                        nc.tensor.wait_ge(*w)
                                        first[0] = False
                                    nc.tensor.matmul(
                                        ps[:], lhsT=gh[:, colg * F:(colg + 1) * F],
                                        rhs=stw[:, colg, :],
                                        start=(mi == 0), stop=(mi == nmm - 1),
                                    )
                                    mi += 1
                            # ---- per-block postprocess (aggT = ps) ----
                            aggT = wpool.tile([F, P], f32, tag="aggT")
                            nc.scalar.activation(aggT[:], ps[:], Act.Copy)
                            mm_ps = ppool.tile([P, G], f32, tag="mmps")
                            nc.tensor.matmul(
                                mm_ps[:], lhsT=aggT[:], rhs=rhs_s[r][:],
                                start=True, stop=True,
                            )
                            root = root1 if r <= 2 else root2
                            t_sb = wpool.tile([P, G], f32, tag="tsb")
                            nc.vector.scalar_tensor_tensor(
                                t_sb[:], mm_ps[:], dcol, root[:, b, :],
                                op0=Alu.mult, op1=Alu.add,
                            )
                            if os.environ.get("GNN_DBG") == f"tsb{r}":
                                nc.sync.dma_start(
                                    out=out_d[b * P:(b + 1) * P, :],
                                    in_=t_sb[:, :OUT_C],
                                )

                            def store_y(yo, rr):
                                if b < NBH:
                                    nc.sync.dma_start(
                                        out=agA[rr][b * P:(b + 1) * P, :], in_=yo[:]
                                    )
                                else:
                                    bb = b - NBH
                                    nc.sync.dma_start(
                                        out=agB[rr][bb * P:(bb + 1) * P, :], in_=yo[:]
                                    )

                            if r == 1:
                                yo = wpool.tile([P, G1], YDT[2], tag="yout")
                                nc.scalar.activation(yo[:], t_sb[:], Act.Relu, scale=dcol)
                                store_y(yo, 2)
                            elif r == 2:
                                out1 = wpool.tile([P, G1], f32, tag="out1")
                                nc.scalar.activation(out1[:], t_sb[:], Act.Relu)
                                hsum = wpool.tile([P, HID_C], f32, tag="hsum")
                                nc.vector.tensor_add(
                                    hsum[:], out1[:, :HID_C], out1[:, HID_C:]
                                )
                                yo = wpool.tile([P, HID_C], f32, tag="yout2")
                                nc.scalar.activation(
                                    yo[:], hsum[:], Act.Copy, scale=dinvh[:, b:b + 1]
                                )
                                store_y(yo, 3)
                                hT_ps = ppool.tile([HID_C, P], f32, tag="tps")
                                nc.tensor.transpose(hT_ps[:], hsum[:], ident[:])
                                hT = wpool.tile([HID_C, P], f32, tag="aggT2")
                                nc.scalar.activation(hT[:], hT_ps[:], Act.Copy)
                                r2_ps = ppool.tile([P, G2], f32, tag="mmps")
                                nc.tensor.matmul(
                                    r2_ps[:], lhsT=hT[:], rhs=rootw2_s[:],
                                    start=True, stop=True,
                                )
                                nc.vector.tensor_add(root2[:, b, :], r2_ps[:], b2rep[:])
                            elif r == 3:
                                yo = wpool.tile([P, G2], f32, tag="yout")
                                nc.scalar.activation(yo[:], t_sb[:], Act.Relu, scale=dcol)
                                store_y(yo, 4)
                            else:
                                ofin = wpool.tile([P, G2], f32, tag="out1")
                                nc.scalar.activation(ofin[:], t_sb[:], Act.Relu)
                                msum = wpool.tile([P, OUT_C], f32, tag="hsum")
                                nc.vector.tensor_add(
                                    msum[:], ofin[:, :OUT_C], ofin[:, OUT_C:]
                                )
                                yo = wpool.tile([P, OUT_C], f32, tag="yout")
                                nc.scalar.activation(yo[:], msum[:], Act.Copy, scale=0.5)
                                nc.sync.dma_start(
                                    out=out_d[b * P:(b + 1) * P, :], in_=yo[:]
                                )
                        # fire AG_A as soon as the half-A output shard exists
                        if b1 == NBH and r < 4 and r < max_round and not skip_ag:
                            nc.gpsimd.collective_compute(
                                "AllGather", Alu.bypass, replica_groups=rg,
                                ins=[agA[r + 1].ap().opt()],
                                outs=[yA[r + 1].ap().opt()],
                            )
                    if r < 4 and r < max_round and not skip_ag:
                        nc.gpsimd.collective_compute(
                            "AllGather", Alu.bypass, replica_groups=rg,
                            ins=[agB[r + 1].ap().opt()], outs=[yB[r + 1].ap().opt()],
                        )

    nc.compile()
    return nc


def build_in_maps(inputs: dict, cfg: Cfg, st: Struct) -> list:
    x = np.asarray(inputs["x"], dtype=np.float32)
    wmap = build_weight_inputs(inputs, cfg)
    in_maps = []
    for c in range(cfg.CORES):
        xs = np.zeros((cfg.NPC, cfg.IN_C), np.float32)
        mine = np.nonzero(st.pid // cfg.NPC == c)[0]
        loc = st.pid[mine] % cfg.NPC
        xs[loc] = x[mine]
        m = {
            "xs": xs,
            "idx16": np.ascontiguousarray(st.idx16[c]),
            "stair": np.ascontiguousarray(st.stair[c]),
            "dinv": np.ascontiguousarray(st.dinv_pc[c]),
        }
        m.update(wmap)
        in_maps.append(m)
    return in_maps


def assemble_output(results: list, cfg: Cfg, st: Struct) -> np.ndarray:
    full = np.concatenate(
        [np.asarray(results[c]["out"]) for c in range(cfg.CORES)], axis=0
    )
    return np.ascontiguousarray(full[st.pid]).astype(np.float32)


def kernel(**inputs) -> np.ndarray:
    from concourse.bass_utils import run_bass_kernel_spmd

    cfg = Cfg()
    st = build_structure(np.asarray(inputs["edge_index"]), cfg)
    nc = build_nc(cfg, st)
    in_maps = build_in_maps(inputs, cfg, st)
    res = run_bass_kernel_spmd(nc, in_maps, core_ids=list(range(cfg.CORES)))
    return assemble_output(res.results, cfg, st)


if __name__ == "__main__":
    pass
